# revision 3
# baseline (speedup 1.0000x reference)
"""Chamfer distance kernel for Trainium2 (8 NeuronCores, SPMD).

Problem: B=16 batches of two 4096-point 3D clouds; cost =
  sum_b 0.5*(mean_n min_m d2[b,n,m] + mean_m min_n d2[b,n,m]).

Sharding: data-parallel over batch. Each of the 8 cores handles 2 batches
and does the full 4096x4096 pairwise search for both directions.

Device algorithm (per core):
  - The pairwise distance matrix is produced directly by the PE as a K=16
    matmul: coordinates are split into bf16 hi/lo pairs (~2^-18 relative
    precision, i.e. fp32-like) and the squared norms ride along as extra
    contraction rows, so a single matmul emits distance values into PSUM
    at full bf16 rate.
  - Both reduction directions are row-min problems (the second direction is
    a transposed matmul pass), so all mins are free-axis reductions.
  - MODE "plain": VectorE tensor_reduce(min) straight from PSUM (baseline;
    VectorE consumes 1 element/lane/cycle).
  - MODE "max2": matmuls emit NEGATED distances; a runtime-registered
    custom DVE op MAX2_REDUCE_ANT (out = max(in0, in1), accum_out =
    max(seed, max_k out)) consumes a PSUM chunk on the PSUM port and a
    ScalarE-made SBUF copy of a second chunk on the SBUF port
    simultaneously -- 2 elements/lane/cycle through VectorE, with the
    row-max seeded across chunks. Host negates at the end.
  - Row minima/maxima land in a [128, 32] tile per (batch, direction),
    DMA'd out; host computes means and the final scalar.
"""

import sys

sys.path.insert(0, "/opt/trn_rl_repo")

from contextlib import ExitStack

import ml_dtypes
import numpy as np

import concourse.bass as bass  # noqa: F401
import concourse.tile as tile
from concourse import bacc, mybir
from concourse.bass_utils import run_bass_kernel_spmd

B, N, D = 16, 4096, 3
NCORES = 8
BPC = B // NCORES  # batches per core
K = 16  # augmented contraction rows
NT = N // 128  # 32 row tiles
AVG_SCALE = 0.5
BF16 = ml_dtypes.bfloat16
BIGF = 3.0e38
F32 = mybir.dt.float32
MIN = mybir.AluOpType.min

MODE = "max2"  # "plain" | "max2"

_NC = {}


def _register_max2():
    """Register the MAX2_REDUCE_ANT custom DVE op (idempotent)."""
    import concourse.dve_ops as dops
    from concourse.dve_spec import C0, Spec, Src0, Src1, _has_src1, lower, maxx

    for op in dops.OPS:
        if op.name == "MAX2_REDUCE_ANT":
            return op

    def _ref(in0, in1, c0, c1, c2):
        b = np.maximum(np.asarray(in0, np.float32),
                       np.asarray(in1, np.float32)).astype(np.float32)
        acc = np.maximum(b.reshape(b.shape[0], -1).max(-1, keepdims=True),
                         np.asarray(c0, np.float32))
        return b, acc.astype(np.float32)

    spec = Spec(body=maxx(Src0, Src1), accum=maxx, accum_init=C0,
                reference=_ref)
    name = "MAX2_REDUCE_ANT"
    row = dops._CUSTOM_DVE_ROW_BASE + len(dops.OPS)
    assert row < 0x20
    dops._SUB_OPCODE_FOR_NAME[name] = row
    shas = {}
    for ver in ("v3", "v4"):
        spec_l = dops.DveOpSpec(name=name, opcode=row,
                                uops=lower(spec, ver=ver),
                                rd1_en=_has_src1(spec))
        shas[ver] = spec_l.sha(ver)
    op = dops.DveOp(name, spec, subdim=False, uops_sha=shas)
    dops.OPS.append(op)
    dops.CUSTOM_DVE_SPECS[name] = spec
    return op


def _build(mode):
    nc = bacc.Bacc("TRN2", target_bir_lowering=False, debug=False)
    aug = nc.dram_tensor("aug", [BPC, K, 4 * N], mybir.dt.bfloat16,
                         kind="ExternalInput").ap()
    out = nc.dram_tensor("out", [BPC, 2, 128, NT], F32,
                         kind="ExternalOutput").ap()
    max2 = _register_max2() if mode == "max2" else None

    with tile.TileContext(nc) as tc, ExitStack() as ctx:
        inpool = ctx.enter_context(tc.tile_pool(name="inp", bufs=2))
        statp = ctx.enter_context(tc.tile_pool(name="stat", bufs=2))
        tmpp = ctx.enter_context(tc.tile_pool(name="tmp", bufs=2))
        if mode == "plain":
            psp = ctx.enter_context(tc.tile_pool(name="ps", bufs=2,
                                                 space="PSUM"))
        else:
            psA = ctx.enter_context(tc.tile_pool(name="psA", bufs=2,
                                                 space="PSUM"))
            psB = ctx.enter_context(tc.tile_pool(name="psB", bufs=2,
                                                 space="PSUM"))
            cpp = ctx.enter_context(tc.tile_pool(name="cp", bufs=3))
            dummyp = ctx.enter_context(tc.tile_pool(name="dummy", bufs=1))
            dummy = dummyp.tile([128, 1], F32, tag="dummy")

        for b in range(BPC):
            mats = inpool.tile([K, 4 * N], mybir.dt.bfloat16, tag="mats")
            nc.sync.dma_start(mats[:], aug[b])
            for d in range(2):
                lbase = (0 if d == 0 else 2) * N
                rbase = (1 if d == 0 else 3) * N
                rowext = statp.tile([128, NT], F32, tag="rowext")
                for nt in range(NT):
                    lhsT = mats[:, lbase + nt * 128:lbase + (nt + 1) * 128]
                    if mode == "plain":
                        cc = tmpp.tile([128, 2], F32, tag="cc")
                        for h in range(2):
                            p = psp.tile([128, 2048], F32, tag="p")
                            cb = rbase + h * 2048
                            for j in range(4):
                                nc.tensor.matmul(
                                    p[:, j * 512:(j + 1) * 512], lhsT,
                                    mats[:, cb + j * 512:cb + (j + 1) * 512],
                                    start=True, stop=True)
                            nc.vector.tensor_reduce(
                                cc[:, h:h + 1], p[:], op=MIN,
                                axis=mybir.AxisListType.X)
                        view = cc.rearrange("p (a b) -> p a b", b=2)
                        nc.vector.tensor_reduce(
                            rowext[:, nt:nt + 1], view, op=MIN,
                            axis=mybir.AxisListType.X)
                    else:
                        rtmp = tmpp.tile([128, 1], F32, tag="rtmp")
                        for h in range(2):
                            # chunk of 2048 cols: 1024 -> psB (ScalarE copies
                            # to SBUF), 1024 -> psA (VectorE PSUM port)
                            pb = psB.tile([128, 1024], F32, tag="pb")
                            pa = psA.tile([128, 1024], F32, tag="pa")
                            cb = rbase + h * 2048
                            for j in range(2):
                                nc.tensor.matmul(
                                    pb[:, j * 512:(j + 1) * 512], lhsT,
                                    mats[:, cb + j * 512:cb + (j + 1) * 512],
                                    start=True, stop=True)
                            for j in range(2):
                                cb2 = cb + 1024
                                nc.tensor.matmul(
                                    pa[:, j * 512:(j + 1) * 512], lhsT,
                                    mats[:, cb2 + j * 512:cb2 + (j + 1) * 512],
                                    start=True, stop=True)
                            cp = cpp.tile([128, 1024], F32, tag="cp")
                            nc.scalar.copy(cp[:], pb[:])
                            nc.vector._custom_dve(
                                max2, out=dummy.broadcast_to(pa.shape),
                                in0=pa[:], in1=cp[:],
                                s0=(-BIGF if h == 0 else rtmp[:]),
                                accum_out=(rtmp[:] if h == 0
                                           else rowext[:, nt:nt + 1]))
                nc.sync.dma_start(out[b, d], rowext[:])

    nc.compile()
    return nc


def get_nc(mode=None):
    mode = mode or MODE
    if mode not in _NC:
        _NC[mode] = _build(mode)
    return _NC[mode]


def _split_bf16(v):
    """fp32 array -> (hi, lo) fp32 arrays that are exactly bf16 values."""
    h = v.astype(BF16)
    l = (v - h.astype(np.float32)).astype(BF16)
    return h.astype(np.float32), l.astype(np.float32)


def _aug_mats(pts, sign):
    """pts [N, 3] fp32 -> (Lmat, Rmat) [K, N] bf16 with sign=+1 for +d2
    (plain/min mode) or -1 for -d2 (max2 mode).

    Contraction pairing (L row, R row), sign s:
      per coord c: (h,h,l,l) x (-2sh,-2sl,-2sh,-2sl) -> -2s*c_a*c_b
      rows 12-13: (sq_h, sq_l) x (s, s)              -> s*|a|^2
      rows 14-15: (1, 1) x (s*sq_h, s*sq_l)          -> s*|b|^2
    """
    n = pts.shape[0]
    s = float(sign)
    sq = np.sum(pts.astype(np.float64) ** 2, axis=-1).astype(np.float32)
    sqh, sql = _split_bf16(sq)
    ones = np.ones(n, np.float32)
    lrows, rrows = [], []
    for c in range(3):
        h, l = _split_bf16(pts[:, c])
        lrows += [h, h, l, l]
        rrows += [-2.0 * s * h, -2.0 * s * l, -2.0 * s * h, -2.0 * s * l]
    lrows += [sqh, sql, ones, ones]
    rrows += [s * ones, s * ones, s * sqh, s * sql]
    return np.stack(lrows).astype(BF16), np.stack(rrows).astype(BF16)


def _prep_inputs(points1, points2, mode=None):
    mode = mode or MODE
    sign = 1.0 if mode == "plain" else -1.0
    p1 = np.asarray(points1, dtype=np.float32)
    p2 = np.asarray(points2, dtype=np.float32)
    aug = np.empty((B, K, 4 * N), dtype=BF16)
    for b in range(B):
        l1, r1 = _aug_mats(p1[b], sign)
        l2, r2 = _aug_mats(p2[b], sign)
        aug[b, :, 0 * N:1 * N] = l1  # dir0 lhsT: points1 rows
        aug[b, :, 1 * N:2 * N] = r2  # dir0 rhs:  points2 cols
        aug[b, :, 2 * N:3 * N] = l2  # dir1 lhsT: points2 rows
        aug[b, :, 3 * N:4 * N] = r1  # dir1 rhs:  points1 cols
    return [{"aug": aug[c * BPC:(c + 1) * BPC]} for c in range(NCORES)]


def _assemble(results, mode=None):
    mode = mode or MODE
    sgn = 1.0 if mode == "plain" else -1.0
    total = 0.0
    for c in range(NCORES):
        r = results[c]["out"]
        for b in range(BPC):
            m1 = sgn * float(np.mean(r[b, 0].astype(np.float64)))
            m2 = sgn * float(np.mean(r[b, 1].astype(np.float64)))
            total += AVG_SCALE * (m1 + m2)
    return np.asarray(total, dtype=np.float32)


def run(points1, points2, trace=False, tmpdir=None, mode=None):
    mode = mode or MODE
    nc = get_nc(mode)
    in_maps = _prep_inputs(points1, points2, mode)
    res = run_bass_kernel_spmd(nc, in_maps, list(range(NCORES)),
                               trace=trace, tmpdir=tmpdir)
    return _assemble(res.results, mode), res


def kernel(points1, points2):
    out, _ = run(points1, points2)
    return out
